# revision 78
# baseline (speedup 1.0000x reference)
"""HRR adapted attention kernel for 8 trn2 cores.

Math (verified vs reference in f64):
  q,k,v = h @ W{q,k,v}.T + b      (per-row, D=2048)
  Qf = rfft(q); Kf = rfft(k)/(|rfft(k)|+eps); Vf likewise
  Mf = causal-cumsum_S(Kf*Vf);  Of = conj(Qf)*Mf;  adapter = irfft(Of)
  out = base + gate*adapter

All FFTs become matmuls: the DFT folds into the projections,
G = W.T @ [C|S] in [d,f] orientation, so the Q/K/V spectra come straight
out of hT.T @ G in a freq-on-partition layout where the causal cumsum is
a native tensor_tensor_scan along the free (sequence) dim.

Sharding: 2 batch groups x 4 spectrum shards. Core c handles batch c//4
and 256 packed rfft bins (2 f-tiles of 128). Each core folds only its own
spectrum slice (full W needed, no fold collective; a radix-2 split over
the e-contraction via host-built W+ / W- halves the fold matmul time),
projects / binds / scans all 4096 rows of its batch locally in an
s-chunked software pipeline, and computes a PARTIAL inverse DFT over its
f-slice chunk by chunk. Grouped bf16 ReduceScatters over s-pieces
(2+2+2+1+1 chunks) combine the partials - the early pieces fully overlap
the compute pipeline and only the last single-chunk piece sits in the
tail. The output is prefilled with base via paced DRAM-to-DRAM copies and
each reduced piece lands with one casting accumulate-DMA, so the epilogue
needs no SBUF staging and no on-chip transposes exist anywhere (h arrives
pre-transposed, base/out live in the transposed [d, s] layout).

The packed spectrum keeps rfft bins DC and Nyquist in the re/im planes of
packed column 0 (both real). Their special normalize/bind/unbind algebra
is expressed uniformly via per-partition {0,1} mask columns, and the
radix-2 even/odd bin ordering is a fixed within-tile permutation absorbed
entirely by host-side row permutation of the bias/mask/iDFT constants, so
the SPMD program is identical on every core.
"""

import numpy as np

import concourse.bass as bass
import concourse.mybir as mybir
import concourse.tile as tile
from concourse import bacc, bass_utils

F32 = mybir.dt.float32
BF16 = mybir.dt.bfloat16
AX = mybir.AxisListType
ALU = mybir.AluOpType
ACTF = mybir.ActivationFunctionType

B, S, D = 2, 4096, 2048
N_CORES = 8
NG, GS = 2, 4              # batch groups x spectrum shards
FP = 1024                  # packed rfft bins (col0: re=DC, im=Nyquist)
FBLK = FP // GS            # 256 packed bins per core
NFT = FBLK // 128          # 2 local f-tiles
ND = D // 128              # 16 d tiles
NE = D // 128              # 16 e tiles
NE2 = D // 256             # 8 e tiles after the radix-2 split
DQ = D // GS               # 512 output d rows per core
SCH = 512                  # sequence chunk for the pipeline
NSC = S // SCH             # 8 chunks
EPS = 1e-8
# mat order: (name, use_sin(ci), w_idx, bias_col)
MATS = [("kre", 0, 1, 2), ("kim", 1, 1, 3),
        ("vre", 0, 2, 4), ("vim", 1, 2, 5),
        ("qre", 0, 0, 0), ("qim", 1, 0, 1)]

_CACHE = {}


def _build():
    nc = bacc.Bacc("TRN2", target_bir_lowering=False, debug=False,
                   enable_asserts=False, num_devices=N_CORES)

    hT_in = nc.dram_tensor("ht", [D, S], BF16, kind="ExternalInput").ap()
    # radix-2 folded weights, pre-tiled host-side into the SBUF image per
    # d-block: wp = W[:D/2] + W[D/2:] (even rfft bins), wm = difference
    # (odd bins); wp[dt, p, t*128+c] = Wp[t*128+p, dt*128+c]
    wpm_ins = [nc.dram_tensor(f"wpm{x}", [ND * 128, 2 * NE2 * 128], BF16,
                              kind="ExternalInput").ap() for x in "qkv"]
    cev_in = nc.dram_tensor("cev", [D // 2, 256], BF16, kind="ExternalInput").ap()
    cod_in = nc.dram_tensor("cod", [D // 2, 256], BF16, kind="ExternalInput").ap()
    am_in = nc.dram_tensor("am2", [FBLK, D], BF16, kind="ExternalInput").ap()
    bm_in = nc.dram_tensor("bm2", [FBLK, D], BF16, kind="ExternalInput").ap()
    bfc_in = nc.dram_tensor("bfc", [128, NFT * 6], F32, kind="ExternalInput").ap()
    # cols: mz_ft0, mn_ft0, mz_ft1, mn_ft1, gate
    mz_in = nc.dram_tensor("mzg", [128, 5], F32, kind="ExternalInput").ap()
    baseT_in = nc.dram_tensor("baseT", [DQ, S], F32, kind="ExternalInput").ap()
    outT = nc.dram_tensor("outT", [DQ, S], F32, kind="ExternalOutput").ap()

    with nc.allow_low_precision("bf16 spectra; scan state stays fp32"), \
         tile.TileContext(nc) as tc, \
         tc.tile_pool(name="pc", bufs=1) as PC, \
         tc.tile_pool(name="dram", bufs=1, space="DRAM") as DR:

        # ---------- constants (act queue; W leads the SP queue) ----------
        mz_sb = PC.tile([128, 5], F32, tag="mz")
        nc.scalar.dma_start(mz_sb[:], mz_in[:])
        bfc_sb = PC.tile([128, NFT * 6], F32, tag="bfc")
        nc.scalar.dma_start(bfc_sb[:], bfc_in[:])
        eps_sb = PC.tile([128, 1], F32, tag="eps")
        nc.vector.memset(eps_sb[:], EPS ** 4)   # bias for the product-rsqrt
        zeros_bf = PC.tile([128, SCH], BF16, tag="zer")
        nc.vector.memset(zeros_bf[:], 0.0)

        # ---------- DRAM intermediates ----------
        # pieces of the s-axis; later pieces are single chunks so the last
        # collective (the only unoverlapped one) is as small as possible
        PIECES = [(0, 1), (2, 3), (4, 5), (6,), (7,)]
        PIDX = {sc: p for p, chs in enumerate(PIECES) for sc in chs}
        POFF = {sc: (sc - chs[0]) * SCH
                for chs in PIECES for sc in chs}
        PCOL = [chs[0] * SCH for chs in PIECES]
        parts = [DR.tile([D, len(chs) * SCH], BF16, tag=f"pa{p}",
                         name=f"pa{p}") for p, chs in enumerate(PIECES)]
        rss = [DR.tile([DQ, len(chs) * SCH], BF16, tag=f"rs{p}",
                       name=f"rs{p}") for p, chs in enumerate(PIECES)]

        # iDFT mats + main pipeline
        with tc.tile_pool(name="pgl", bufs=1) as PGL:
            ab = {}

            # ============ fold: G[d,f-slice] = W.T @ [C|S] ============
            with tc.tile_pool(name="pgG", bufs=1) as PGG, \
                 tc.tile_pool(name="pht", bufs=2) as PHT:

                def load_htc(sc):
                    htc = PHT.tile([128, NE * SCH], BF16, tag="htc")
                    nc.sync.dma_start(
                        htc[:].rearrange("p (t s) -> p t s", s=SCH),
                        hT_in[:, sc * SCH:(sc + 1) * SCH]
                        .rearrange("(t p) s -> p t s", p=128))
                    return htc

                G = [PGG.tile([128, ND * 512], BF16, tag=f"G{wi}",
                              name=f"G{wi}") for wi in range(3)]
                with tc.tile_pool(name="pf", bufs=16) as PF, \
                     tc.tile_pool(name="psf", bufs=1, space="PSUM") as PPF:
                    # DFT bases (ev/od by bin parity) via the act queue so
                    # the W loads lead the SP queue; first e-tile lands
                    # separately so the first fold matmuls start early
                    cev_sb = PF.tile([128, NE2 * 256], BF16, tag="cev",
                                     bufs=1)
                    cod_sb = PF.tile([128, NE2 * 256], BF16, tag="cod",
                                     bufs=1)
                    for cs_sb, cs_in in ((cev_sb, cev_in), (cod_sb, cod_in)):
                        nc.scalar.dma_start(
                            cs_sb[:].rearrange("p (t c) -> p t c", c=256),
                            cs_in.rearrange("(t p) c -> p t c", p=128))
                    htc0 = PHT.tile([128, NE * SCH], BF16, tag="htc")
                    for wi in range(3):
                        for dt in range(ND):
                            if wi == 2 and dt in (2, 5, 8, 11):
                                # paced quarters so no single transfer
                                # starves the fold's W feed
                                qq = (2, 5, 8, 11).index(dt)
                                nc.scalar.dma_start(
                                    htc0[:, qq * 4 * SCH:(qq + 1) * 4 * SCH]
                                    .rearrange("p (t s) -> p t s", s=SCH),
                                    hT_in[qq * 512:(qq + 1) * 512, 0:SCH]
                                    .rearrange("(t p) s -> p t s", p=128))
                            wpm_sb = PF.tile([128, 2 * NE2 * 128], BF16,
                                             tag="wpm")
                            nc.sync.dma_start(
                                wpm_sb[:],
                                wpm_ins[wi][dt * 128:(dt + 1) * 128, :])
                            wp_sb = wpm_sb[:, :NE2 * 128]
                            wm_sb = wpm_sb[:, NE2 * 128:]
                            psf = PPF.tile([128, 512], F32, tag=f"pf{dt % 2}")
                            for blk in range(4):
                                for par, (wz, cz) in enumerate(
                                        ((wp_sb, cev_sb), (wm_sb, cod_sb))):
                                    c0 = blk * 128 + par * 64
                                    for e in range(NE2):
                                        nc.tensor.matmul(
                                            psf[:, c0:c0 + 64],
                                            wz[:, e * 128:(e + 1) * 128],
                                            cz[:, e * 256 + blk * 64:
                                               e * 256 + (blk + 1) * 64],
                                            start=(e == 0),
                                            stop=(e == NE2 - 1))
                            nc.scalar.copy(
                                G[wi][:, dt * 512:(dt + 1) * 512], psf[:])
                    for ftl in range(NFT):
                        amt = PGL.tile([128, D], BF16, tag=f"am{ftl}",
                                       name=f"am{ftl}")
                        nc.sync.dma_start(
                            amt[:], am_in[ftl * 128:(ftl + 1) * 128, :])
                        bmt = PGL.tile([128, D], BF16, tag=f"bm{ftl}",
                                       name=f"bm{ftl}")
                        nc.sync.dma_start(
                            bmt[:], bm_in[ftl * 128:(ftl + 1) * 128, :])
                        ab[ftl] = (amt, bmt)

                # ============ s-chunk pipeline ============
                with tc.tile_pool(name="pm", bufs=2) as PM, \
                     tc.tile_pool(name="psm", bufs=1, space="PSUM") as PPM:
                    m_prev = {}
                    ofv = {}

                    def irfft_sc(sc):
                        # full-depth partial inverse DFT for one s-chunk
                        part = parts[PIDX[sc]]
                        pcol = POFF[sc]
                        for dt in range(ND):
                            psi = PPM.tile([128, SCH], F32,
                                           tag=f"ir{dt % 4}")
                            step = 0
                            for ftl in range(NFT):
                                amt, bmt = ab[ftl]
                                for pi, abt in ((0, amt), (1, bmt)):
                                    nc.tensor.matmul(
                                        psi[:],
                                        abt[:, dt * 128:(dt + 1) * 128],
                                        ofv[(sc, ftl, pi)],
                                        start=(step == 0),
                                        stop=(step == 2 * NFT - 1))
                                    step += 1
                            # drain on DVE (PE-driven, no act-queue
                            # bubbles), folding the gate scale in. The last
                            # two chunks drain on the idle act engine instead
                            # so the final chunk's unbind chain is not queued
                            # behind them on the DVE.
                            stg = PM.tile([128, SCH], BF16, tag=f"sta{dt % 4}")
                            if sc >= NSC - 2:
                                nc.scalar.activation(
                                    stg[:], psi[:], ACTF.Identity,
                                    scale=mz_sb[:, 4:5])
                            else:
                                nc.vector.tensor_scalar_mul(
                                    stg[:], psi[:], mz_sb[:, 4:5])
                            nc.sync.dma_start(
                                part[dt * 128:(dt + 1) * 128,
                                     pcol:pcol + SCH], stg[:])

                    def rs_q(q):
                        nc.gpsimd.collective_compute(
                            "ReduceScatter", ALU.add,
                            replica_groups=[[0, 1, 2, 3], [4, 5, 6, 7]],
                            ins=[parts[q].opt()], outs=[rss[q].opt()])

                    def accum_q(q):
                        # epilogue: one accumulate-DMA folds the (already
                        # gate-scaled) reduced piece into the base-prefilled
                        # output; casts bf16 -> f32 in the DGE. Emitted only
                        # once rs_q(q) is long done so the wait never blocks
                        # Pool's in-order queue.
                        pl = len(PIECES[q]) * SCH
                        nc.gpsimd.dma_start(
                            outT[:, PCOL[q]:PCOL[q] + pl], rss[q][:, :],
                            accum_op=ALU.add)

                    htc = htc0
                    for sc in range(NSC):
                        s0, s1 = sc * SCH, (sc + 1) * SCH
                        htc_next = load_htc(sc + 1) if sc + 1 < NSC else None
                        # an older chunk's partial iDFT keeps PE busy while
                        # the DVE works through the recent chunks' planes
                        if sc > 1:
                            j = sc - 2
                            irfft_sc(j)
                            if j == PIECES[PIDX[j]][-1]:
                                rs_q(PIDX[j])       # piece complete
                        planes = {}
                        for mi, (_, ci, wi, bcol) in enumerate(MATS):
                            for ftl in range(NFT):
                                ps = PPM.tile([128, SCH], F32,
                                              tag=f"pp{(2 * mi + ftl) % 3}")
                                off = ci * 256 + ftl * 128
                                for dt in range(ND):
                                    nc.tensor.matmul(
                                        ps[:],
                                        G[wi][:, dt * 512 + off:
                                              dt * 512 + off + 128],
                                        htc[:, dt * SCH:(dt + 1) * SCH],
                                        start=(dt == 0), stop=(dt == ND - 1))
                                pl = PM.tile([128, SCH], BF16,
                                             tag=f"pl{mi}_{ftl}")
                                nc.scalar.activation(
                                    pl[:], ps[:], ACTF.Identity,
                                    bias=bfc_sb[:, ftl * 6 + bcol:
                                                ftl * 6 + bcol + 1])
                                planes[(mi, ftl)] = pl
                        if sc < len(PIECES):
                            # prefill this piece of the output with base.
                            # act HWDGE: paced by the act queue's psum waits
                            # (can't race ahead into the fold's DMA window)
                            # and the SEQ is released before the transfer;
                            # two halves so partial stores never queue far
                            # behind it on the DMA pipe
                            pl_ = len(PIECES[sc]) * SCH
                            c0 = PCOL[sc]
                            nq = 2 * len(PIECES[sc])
                            for hh in range(nq):
                                nc.scalar.dma_start(
                                    outT[:, c0 + hh * pl_ // nq:
                                         c0 + (hh + 1) * pl_ // nq],
                                    baseT_in[:, c0 + hh * pl_ // nq:
                                             c0 + (hh + 1) * pl_ // nq])
                        for ftl in range(NFT):
                            mz = mz_sb[:, 2 * ftl:2 * ftl + 1]
                            mn = mz_sb[:, 2 * ftl + 1:2 * ftl + 2]
                            kre, kim = planes[(0, ftl)], planes[(1, ftl)]
                            vre, vim = planes[(2, ftl)], planes[(3, ftl)]
                            qre, qim = planes[(4, ftl)], planes[(5, ftl)]
                            # --- unit-magnitude norms (masked for the two
                            # real bins packed in partition 0 of ft 0).
                            # scale_re = rsqrt(|K|^2 |V|^2) needs only one
                            # sqrt per re/im pair ---
                            rr = {}
                            for pj, (re_, im_) in enumerate(((kre, kim),
                                                            (vre, vim))):
                                sq0 = PM.tile([128, SCH], BF16, tag=f"sq0{pj}")
                                sq1 = PM.tile([128, SCH], BF16, tag=f"sq1{pj}")
                                nc.scalar.square(sq0[:], re_[:])
                                nc.scalar.square(sq1[:], im_[:])
                                ra = PM.tile([128, SCH], BF16, tag=f"ra{pj}")
                                rb = PM.tile([128, SCH], BF16, tag=f"rb{pj}")
                                nc.vector.scalar_tensor_tensor(
                                    ra[:], sq1[:], mz, sq0[:],
                                    ALU.mult, ALU.add)
                                nc.vector.scalar_tensor_tensor(
                                    rb[:], sq0[:], mz, sq1[:],
                                    ALU.mult, ALU.add)
                                rr[pj] = (ra, rb)
                            kra, krb = rr[0]
                            vra, vrb = rr[1]
                            nc.vector.tensor_mul(kra[:], kra[:], vra[:])
                            nc.vector.tensor_mul(krb[:], krb[:], vrb[:])
                            nc.scalar.activation(kra[:], kra[:], ACTF.Sqrt,
                                                 bias=eps_sb[:])
                            nc.scalar.activation(krb[:], krb[:], ACTF.Sqrt,
                                                 bias=eps_sb[:])
                            nc.vector.reciprocal(kra[:], kra[:])
                            nc.vector.reciprocal(krb[:], krb[:])
                            # --- bind: cre+i*cim = Kn * Vn (masked) ---
                            u0 = PM.tile([128, SCH], BF16, tag="u0")
                            u1 = PM.tile([128, SCH], BF16, tag="u1")
                            t0 = PM.tile([128, SCH], BF16, tag="t0")
                            t1 = PM.tile([128, SCH], BF16, tag="t1")
                            cre = PM.tile([128, SCH], BF16, tag="cre")
                            cim = PM.tile([128, SCH], BF16, tag="cim")
                            nc.vector.tensor_mul(u0[:], kre[:], vre[:])
                            nc.vector.tensor_mul(u1[:], kim[:], vim[:])
                            nc.vector.scalar_tensor_tensor(
                                cre[:], u1[:], mn, u0[:], ALU.mult, ALU.add)
                            nc.vector.tensor_mul(t0[:], kre[:], vim[:])
                            nc.vector.tensor_mul(t1[:], kim[:], vre[:])
                            nc.vector.tensor_add(t0[:], t0[:], t1[:])
                            nc.vector.tensor_sub(t0[:], t0[:], u1[:])
                            nc.vector.scalar_tensor_tensor(
                                cim[:], t0[:], mz, u1[:], ALU.mult, ALU.add)
                            nc.vector.tensor_mul(cre[:], cre[:], kra[:])
                            nc.vector.tensor_mul(cim[:], cim[:], krb[:])
                            # --- causal scan (fp32 state, bf16 carry) ---
                            ms = []
                            for pi, cv in enumerate((cre, cim)):
                                mt = PM.tile([128, SCH], BF16,
                                             tag=f"m{ftl}{pi}")
                                init = (0.0 if sc == 0
                                        else m_prev[(ftl, pi)][:, SCH - 1:SCH])
                                nc.vector.tensor_tensor_scan(
                                    mt[:], cv[:], zeros_bf[:], init,
                                    ALU.add, ALU.add)
                                m_prev[(ftl, pi)] = mt
                                ms.append(mt)
                            # --- unbind: Of = conj(Qf) * Mf (masked) ---
                            # per-chunk tiles (bufs=3 covers the iDFT lag);
                            # slicing one big tile would serialize the iDFT
                            # behind later chunks' unbind writes
                            o0 = PM.tile([128, SCH], BF16,
                                         tag=f"of{ftl}0", bufs=3)
                            o1 = PM.tile([128, SCH], BF16,
                                         tag=f"of{ftl}1", bufs=3)
                            ofv[(sc, ftl, 0)] = o0[:]
                            ofv[(sc, ftl, 1)] = o1[:]
                            nc.vector.tensor_mul(u0[:], qre[:], ms[0][:])
                            nc.vector.tensor_mul(u1[:], qim[:], ms[1][:])
                            nc.vector.scalar_tensor_tensor(
                                ofv[(sc, ftl, 0)], u1[:], mz, u0[:],
                                ALU.mult, ALU.add)
                            nc.vector.tensor_mul(t0[:], qre[:], ms[1][:])
                            nc.vector.tensor_mul(t1[:], qim[:], ms[0][:])
                            nc.vector.tensor_sub(t0[:], t0[:], t1[:])
                            nc.vector.tensor_sub(t0[:], t0[:], u1[:])
                            nc.vector.scalar_tensor_tensor(
                                ofv[(sc, ftl, 1)], t0[:], mz, u1[:],
                                ALU.mult, ALU.add)
                        htc = htc_next
                    for j in (NSC - 2, NSC - 1):
                        irfft_sc(j)
                        if j == PIECES[PIDX[j]][-1]:
                            rs_q(PIDX[j])
                    # all epilogue accums after the last collective emission:
                    # the early ones transfer while rs4 is still reducing,
                    # and nothing queues behind them on Pool
                    for p in range(len(PIECES)):
                        accum_q(p)

    nc.compile()
    return nc


def _constants():
    npbf = mybir.dt.np(BF16)
    e = np.arange(D, dtype=np.float64)
    f = np.arange(FP, dtype=np.float64)
    ang = 2.0 * np.pi * np.outer(e, f) / D           # [e, f]
    cp = np.cos(ang)
    sp = -np.sin(ang)
    sp[:, 0] = np.cos(np.pi * e)                     # Nyquist packed in im col 0
    w = np.full(FP, 2.0)
    w[0] = 1.0
    angA = 2.0 * np.pi * np.outer(f, e) / D          # [f, d]
    am = (w[:, None] / D) * np.cos(angA)
    bm = -(w[:, None] / D) * np.sin(angA)
    bm[0, :] = np.cos(np.pi * e) / D                 # Nyquist inverse row
    return (cp.astype(npbf), sp.astype(npbf),
            am.astype(npbf), bm.astype(npbf))


def _run(inputs, trace=False):
    if "nc" not in _CACHE:
        _CACHE["nc"] = _build()
    nc = _CACHE["nc"]
    npbf = mybir.dt.np(BF16)
    cp, sp, am, bm = _CACHE.setdefault("const", _constants())

    h = np.asarray(inputs["hidden_states"], np.float32).reshape(B, S, D)
    base = np.asarray(inputs["base_output"], np.float32).reshape(B, S, D)
    gate = float(np.asarray(inputs["gate"], np.float32).reshape(-1)[0])

    bf = np.zeros((FP, 6), np.float32)
    for j, bn in enumerate(("bq", "bk", "bv")):
        spec = np.fft.rfft(np.asarray(inputs[bn], np.float64))
        bf[:FP, 2 * j] = spec.real[:FP].astype(np.float32)
        bf[:FP, 2 * j + 1] = spec.imag[:FP].astype(np.float32)
        bf[0, 2 * j + 1] = np.float32(spec.real[FP])

    # radix-2 W combos, pre-tiled into the SBUF image per d-block
    def _tile_w(wf):
        return np.ascontiguousarray(
            wf.reshape(NE2, 128, ND, 128).transpose(2, 1, 0, 3)
            .reshape(ND * 128, NE2 * 128)).astype(npbf)
    wpm = {}
    for x in "qkv":
        wf = np.asarray(inputs[f"W{x}"], np.float32)
        wpm[x] = np.ascontiguousarray(np.concatenate(
            [_tile_w(wf[:D // 2] + wf[D // 2:]),
             _tile_w(wf[:D // 2] - wf[D // 2:])], axis=1))

    hT = [np.ascontiguousarray(h[g].T).astype(npbf) for g in range(NG)]

    # within-tile bin permutation: evens first, then odds (tile bases are
    # even, so the perm is the same for every 128-bin tile)
    PERM = np.concatenate([np.arange(0, 128, 2), np.arange(1, 128, 2)])
    in_maps = []
    for c in range(N_CORES):
        g, r = c // GS, c % GS
        # ev/od DFT bases, cols ordered (ci, ftl) to match the fold psum
        cev = np.empty((D // 2, 256), np.float32)
        cod = np.empty((D // 2, 256), np.float32)
        for ci, basis in enumerate((cp, sp)):
            b64 = np.asarray(basis, np.float32)
            for ftl in range(NFT):
                b0 = r * FBLK + ftl * 128
                k = (2 * ci + ftl) * 64
                cev[:, k:k + 64] = b64[:D // 2, b0:b0 + 128:2]
                cod[:, k:k + 64] = b64[:D // 2, b0 + 1:b0 + 128:2]
        bfc = np.empty((128, NFT * 6), np.float32)
        for ftl in range(NFT):
            bfc[:, ftl * 6:(ftl + 1) * 6] = \
                bf[r * FBLK + ftl * 128 + PERM]
        mzg = np.ones((128, 5), np.float32)
        if r == 0:
            mzg[0, 0] = 0.0          # ft0 partition 0: DC/Nyquist real bins
        mzg[:, 1] = -mzg[:, 0]
        mzg[:, 3] = -mzg[:, 2]
        mzg[:, 4] = gate
        baseT = np.ascontiguousarray(base[g][:, DQ * r:DQ * (r + 1)].T)
        am2 = np.empty((FBLK, D), npbf)
        bm2 = np.empty((FBLK, D), npbf)
        for ftl in range(NFT):
            rows = r * FBLK + ftl * 128 + PERM
            am2[ftl * 128:(ftl + 1) * 128] = am[rows]
            bm2[ftl * 128:(ftl + 1) * 128] = bm[rows]
        in_maps.append({
            "ht": hT[g],
            "wpmq": wpm["q"], "wpmk": wpm["k"], "wpmv": wpm["v"],
            "cev": cev.astype(npbf), "cod": cod.astype(npbf),
            "am2": am2, "bm2": bm2,
            "bfc": bfc, "mzg": mzg, "baseT": baseT,
        })

    res = bass_utils.run_bass_kernel_spmd(
        nc, in_maps, core_ids=list(range(N_CORES)), trace=trace)

    out = np.empty((B, S, D), np.float32)
    for c in range(N_CORES):
        g, r = c // GS, c % GS
        out[g][:, DQ * r:DQ * (r + 1)] = res.results[c]["outT"].T
    return out, res


def kernel(**inputs) -> np.ndarray:
    out, _ = _run(inputs)
    return out
